# revision 46
# baseline (speedup 1.0000x reference)
"""Multi-head attention (B=4, N=2048, D=768, H=12, Dh=64) on 8 TRN2 NeuronCores.

Sharding: core c -> batch b = c//2, query rows half = c%2 (1024 rows each).
Each core computes all 12 heads for its (batch, query-half) against the full
2048-key sequence, so outputs are disjoint and no collective is needed.

The kernel is organized as 6 pipelined "pair phases" (one per head pair).
The ACT engine's exp stream is the bottleneck, so all PE work is arranged
to hide under it:
  - scores: two K=64 row-tiled matmuls (array rows 0-63 = head A, 64-127 =
    head B) run concurrently -> both heads' S^T per k-tile in ~512 cycles.
  - attn@V: two M=64 col-tiled matmuls (array cols 0-63 / 64-127) with each
    head's own P^T moving stream run concurrently.
  - softmax denominators: 4-way col-tiled M=32 matmuls with a (1/32)-valued
    stationary, partials replicated over 32 rows each, then
    gpsimd.partition_all_reduce sums all 128 partitions -> d broadcast.
  - exp: mostly ACT (scalar engine); DVE_K tiles per head go through a
    2-instruction custom DVE op pair computing (1 + t + t^2/2)^2048 via
    repeated squaring (t = 0.125*s/2048), rebalancing ACT vs DVE.
  - V projection is interleaved into the score phases as PE filler.
"""

import numpy as np

import concourse.bass as bass
import concourse.bacc as bacc
import concourse.mybir as mybir
import concourse.tile as tile
from concourse.bass_isa import ReduceOp
from concourse.bass_utils import run_bass_kernel_spmd

N_CORES = 8
B, N, D = 4, 2048, 768
H, DH = 12, 64
NQ = 1024           # query rows per core
COLS = 3 * D        # 2304 qkv columns
DT = D // 128       # 6 partition tiles of the model dim
NT = N // 128       # 16 key tiles
QT_TILES = NQ // 128  # 8 query tiles
NP = DT             # 6 head pairs

F32 = mybir.dt.float32
BF16 = mybir.dt.bfloat16

# k-tiles whose exp runs on the DVE (custom squaring op) instead of ACT
DVE_K = ()


# ---------------- custom DVE exp ----------------
# exp(0.125*s) = u^2048, u = 1 + t + t^2/2 = ((t+1)^2 + 1)/2, t = 0.125*s/2048
# op1: u^8 in 8 ALU stages; op2: ^256 (8 squarings). f32 intermediate.
def _register_exp_ops():
    import concourse.dve_ops as dve_ops
    from concourse.dve_spec import Spec, Src0, C0, C1, One, sq, lower
    from concourse.dve_uop import DveOpSpec
    from concourse.dve_table_gen import dve_ver_for

    def by_name(name):
        for o in dve_ops.OPS:
            if o.name == name:
                return o
        return None

    got = (by_name("EXP2K_BASE_ANT"), by_name("EXP2K_SQ8_ANT"))
    if got[0] is not None:
        return got

    a = sq(Src0 * C0 + One) + One
    body1 = sq(sq(sq(a * C1)))

    def ref1(in0, in1, s0, s1, imm2):
        u = ((in0.astype(np.float64) * s0 + 1.0) ** 2 + 1.0) * s1
        return (u ** 8).astype(np.float32)

    body2 = sq(sq(sq(sq(sq(sq(sq(sq(Src0))))))))

    def ref2(in0, in1, s0, s1, imm2):
        return (in0.astype(np.float64) ** 256).astype(np.float32)

    ver = dve_ver_for("TRN2")
    ops = []
    for name, body, ref in (("EXP2K_BASE_ANT", body1, ref1),
                            ("EXP2K_SQ8_ANT", body2, ref2)):
        spec = Spec(body=body, reference=ref)
        row = max(dve_ops._SUB_OPCODE_FOR_NAME.values()) + 1
        assert row < 0x20
        tmp = DveOpSpec(name=name, opcode=row, uops=lower(spec, ver=ver),
                        rd1_en=False)
        op = dve_ops.DveOp(name, spec, subdim=False,
                           uops_sha={ver: tmp.sha(ver)})
        dve_ops._SUB_OPCODE_FOR_NAME[name] = row
        dve_ops.OPS.append(op)
        dve_ops.CUSTOM_DVE_SPECS[name] = spec
        ops.append(op)
    return tuple(ops)


EXP_BASE, EXP_SQ8 = _register_exp_ops()


def build(debug_taps=False):
    nc = bacc.Bacc("TRN2", target_bir_lowering=False, debug=False,
                   num_devices=N_CORES)

    xT_d = nc.dram_tensor("xT", [D, N], BF16, kind="ExternalInput")
    wqkv_d = nc.dram_tensor("wqkv", [D, COLS], BF16, kind="ExternalInput")
    wout_d = nc.dram_tensor("wout", [D, D], BF16, kind="ExternalInput")
    bias_d = nc.dram_tensor("bias", [128, D], F32, kind="ExternalInput")
    out_d = nc.dram_tensor("out", [NQ, D], F32, kind="ExternalOutput")

    taps = {}
    if debug_taps:
        for name, shape, dt in (
                ("tap_V0", [128, H * DH], BF16),
                ("tap_QT0", [128, NQ], BF16),
                ("tap_KT0", [128, N], BF16),
                ("tap_PTA4", [128, NQ], BF16),
                ("tap_PTA5", [128, NQ], BF16),
                ("tap_R0", [128, NQ], F32),
                ("tap_AOTU0", [128, NQ], BF16),
                ("tap_AOT0", [128, NQ], BF16)):
            taps[name] = nc.dram_tensor(name, shape, dt, kind="ExternalOutput")

    with tile.TileContext(nc) as tc:
        with tc.tile_pool(name="persist", bufs=1) as pp, \
             tc.tile_pool(name="small", bufs=2) as smallp, \
             tc.tile_pool(name="outs", bufs=6) as outsp:

            V = [pp.tile([128, H * DH], BF16, name=f"V{i}", tag=f"V{i}")
                 for i in range(NT)]
            AOT = [pp.tile([128, NQ], BF16, name=f"AOT{i}", tag=f"AOT{i}")
                   for i in range(NP)]
            WOB = pp.tile([128, DT * D], BF16, name="WOB", tag="WOB")
            BIAS = pp.tile([128, D], F32, name="BIAS", tag="BIAS")
            ONES32 = pp.tile([128, 32], BF16, name="ONES32", tag="ONES32")
            # block-diagonal selectors for the denominator combine: pd rows
            # 0-31 = (A,qb0), 32-63 = (B,qb0), 64-95 = (A,qb1), 96-127 =
            # (B,qb1) partials; SELq picks its qb's two 32-row blocks into
            # output halves 0-63 / 64-127
            SELQ = [pp.tile([128, 128], BF16, name=f"SELQ{q}", tag=f"SELQ{q}")
                    for q in range(2)]

            nc.gpsimd.memset(ONES32[:], 1.0 / 32.0)
            for q in range(2):
                nc.gpsimd.memset(SELQ[q][:], 0.0)
                nc.gpsimd.memset(SELQ[q][64 * q:64 * q + 32, 0:64], 1.0)
                nc.gpsimd.memset(SELQ[q][64 * q + 32:64 * q + 64, 64:128], 1.0)

            def load_tail_weights():
                # BIAS/WO are only read by the epilogue; keep their DMAs off
                # the startup critical path (each dma_start costs ~650ns of
                # Sync-engine serialization)
                nc.sync.dma_start(BIAS[:], bias_d.ap())
                nc.sync.dma_start(
                    WOB[:].rearrange("p (a n) -> p a n", n=D),
                    wout_d.ap().rearrange("(a p) n -> p a n", p=128))

            with tc.tile_pool(name="projin", bufs=1) as projin, \
                 tc.tile_pool(name="qk", bufs=1) as qkp, \
                 tc.tile_pool(name="pt", bufs=1) as ptp, \
                 tc.tile_pool(name="scr", bufs=1) as scrp, \
                 tc.tile_pool(name="psA", bufs=1, space="PSUM") as psA, \
                 tc.tile_pool(name="pavd", bufs=1, space="PSUM") as pavd:

                # xT in 4 column chunks so the first projection can start
                # after ~1/4 of the transfer
                xTC = [projin.tile([128, DT * 512], BF16, name=f"xTC{c}",
                                   tag=f"xTC{c}") for c in range(4)]

                def dma_xtc(c):
                    nc.sync.dma_start(
                        xTC[c][:].rearrange("p (a n) -> p a n", n=512),
                        xT_d.ap()[:, c * 512:(c + 1) * 512].rearrange(
                            "(a p) n -> p a n", p=128))

                dma_xtc(0)

                def xT_ap(d, lo, hi):
                    c, off = divmod(lo, 512)
                    assert (hi - 1) // 512 == c
                    return xTC[c][:, d * 512 + off:d * 512 + off + (hi - lo)]

                # persistent-ish per-pair state carried between phases
                PT = {}          # (hp,) -> list of 16 PT tiles of current gen
                AOTU = {}        # ht -> unnormalized pair output tile
                R = {}           # ht -> reciprocal-denominator tile

                # ---- V panel filler steps (3 panels x 16 t-steps) ----
                v_steps = []

                def make_v_panel(vp):
                    co = 2 * D + vp * 256
                    wvB = projin.tile([128, DT * 256], BF16, name="wvB",
                                      tag="wvB", bufs=2)
                    wv = [wvB[:, d * 256:(d + 1) * 256] for d in range(DT)]
                    nc.sync.dma_start(
                        wvB[:].rearrange("p (a n) -> p a n", n=256),
                        wqkv_d.ap()[:, co:co + 256].rearrange(
                            "(a p) n -> p a n", p=128))

                    def step(t):
                        ps = psA.tile([128, 512], F32, name="psA", tag="psA")
                        for d in range(DT):
                            nc.tensor.matmul(
                                ps[:, :256],
                                xT_ap(d, t * 128, (t + 1) * 128),
                                wv[d],
                                start=(d == 0), stop=(d == DT - 1))
                        nc.vector.tensor_copy(
                            V[t][:, vp * 256:(vp + 1) * 256], ps[:, :256])
                    return [lambda t=t: step(t) for t in range(NT)]

                def pump(n):
                    for _ in range(n):
                        if v_steps:
                            v_steps.pop(0)()

                # ---- per-k denominator step: 4 concurrent M=32 matmuls,
                # one per (head, qb), into one shared PSUM bank ----
                def denom_step(PTk, pd, k, n_emitted):
                    for qb in range(2):
                        for hp in range(2):
                            g = 2 * qb + hp
                            kw = ({"tile_position": (0, 96)}
                                  if g == 3 else {})
                            nc.tensor.matmul(
                                pd[32 * g:32 * (g + 1), :],
                                ONES32[:],
                                PTk[k][:, hp * NQ + qb * 512:
                                       hp * NQ + (qb + 1) * 512],
                                start=(n_emitted == 0),
                                stop=(n_emitted == NT - 1), **kw)

                # phase end: sum pd's partitions via block-diag SEL matmuls,
                # reciprocal, store into R[ht]
                def denom_combine(ht, pd):
                    R[ht] = smallp.tile([128, NQ], F32, name=f"R{ht % 2}",
                                        tag=f"R{ht % 2}")
                    pdB = smallp.tile([128, 512], BF16, name="pdB",
                                      tag="pdB", bufs=2)
                    nc.vector.tensor_copy(pdB[:], pd[:])
                    for qb in range(2):
                        qs = slice(qb * 512, (qb + 1) * 512)
                        if qb == 0:
                            pdd = pavd.tile([128, 512], F32, name="pdd",
                                            tag="pd")
                        else:
                            pdd = psA.tile([128, 512], F32, name="psA",
                                           tag="psA")
                        nc.tensor.matmul(pdd[:], SELQ[qb][:], pdB[:],
                                         start=True, stop=True)
                        pdR = smallp.tile([128, 512], F32, name="pdR",
                                          tag="pdR", bufs=2)
                        nc.vector.reciprocal_approx_fast(pdR[:], pdd[:])
                        nc.vector.tensor_copy(R[ht][:, qs], pdR[:])

                # ---- attn@V pair slot for one k (both qb) ----
                def av_step(ht, PTk, po, k, n_emitted):
                    for qb in range(2):
                        for hp in range(2):
                            nc.tensor.matmul(
                                po[qb][hp * 64:(hp + 1) * 64, :],
                                V[k][:, (2 * ht + hp) * DH:
                                     (2 * ht + hp + 1) * DH],
                                PTk[k][:, hp * NQ + qb * 512:
                                       hp * NQ + (qb + 1) * 512],
                                start=(n_emitted == 0),
                                stop=(n_emitted == NT - 1))

                def av_drain(ht, po):
                    AOTU[ht] = smallp.tile([128, NQ], BF16,
                                           name=f"AOTU{ht % 2}",
                                           tag=f"AOTU{ht % 2}")
                    for qb in range(2):
                        qs = slice(qb * 512, (qb + 1) * 512)
                        nc.vector.tensor_copy(AOTU[ht][:, qs], po[qb][:])

                def emit_normalize(ht):
                    for qb in range(2):
                        qs = slice(qb * 512, (qb + 1) * 512)
                        nc.vector.tensor_mul(
                            AOT[ht][:, qs], AOTU[ht][:, qs], R[ht][:, qs])

                # ---- exp for one k (both heads, one [128,2048] op) ----
                def emit_exp(psS, PTtile, k):
                    if k in DVE_K:
                        scr = scrp.tile([128, 2 * NQ], F32, name="scr",
                                        tag="scr")
                        nc.vector._custom_dve(EXP_BASE, out=scr[:],
                                              in0=psS[:],
                                              s0=0.125 / 2048.0, s1=0.5)
                        nc.vector._custom_dve(EXP_SQ8, out=PTtile[:],
                                              in0=scr[:])
                    else:
                        nc.scalar.activation(
                            PTtile[:], psS[:],
                            mybir.ActivationFunctionType.Exp, scale=0.125)

                # ---- Q^T/K^T projection for pair ht as filler steps ----
                QK = {}

                def make_proj(ht, pools=None):
                    QT = qkp.tile([128, NQ], BF16, name="QT", tag="QT",
                                  bufs=2)
                    KTp = qkp.tile([128, N], BF16, name="KTp", tag="KTp",
                                   bufs=2)
                    QK[ht] = (QT, KTp)
                    steps = []
                    nstep = [0]
                    for (dst, co, nn) in ((QT, ht * 128, NQ),
                                          (KTp, D + ht * 128, N)):
                        wpB = projin.tile([128, DT * 128], BF16,
                                          name="wqkB", tag="wqkB", bufs=2)
                        wp = [wpB[:, d * 128:(d + 1) * 128]
                              for d in range(DT)]
                        nc.sync.dma_start(
                            wpB[:].rearrange("p (a n) -> p a n", n=128),
                            wqkv_d.ap()[:, co:co + 128].rearrange(
                                "(a p) n -> p a n", p=128))

                        def nb_step(dst=dst, wp=wp, nb=0):
                            if pools is None:
                                ps = psA.tile([128, 512], F32, name="psA",
                                              tag="psA")
                            else:
                                pool, pname = pools[nstep[0] % len(pools)]
                                nstep[0] += 1
                                ps = pool.tile([128, 512], F32, name=pname,
                                               tag=pname)
                            for d in range(DT):
                                nc.tensor.matmul(
                                    ps[:],
                                    wp[d],
                                    xT_ap(d, nb * 512, (nb + 1) * 512),
                                    start=(d == 0), stop=(d == DT - 1))
                            nc.vector.tensor_copy(
                                dst[:, nb * 512:(nb + 1) * 512], ps[:])
                        for nb in range(nn // 512):
                            steps.append(lambda f=nb_step, nb=nb: f(nb=nb))
                    return steps

                # ================= pair phases =================
                # Everything for pair ht runs inside phase ht's k-loop:
                # scores+exp, then (one step later for DVE-exp'd tiles)
                # attn@V and denominator accumulation. The next pair's
                # projection and the V panels ride as fillers. The exp
                # stream never has to wait for a serial prologue.
                with tc.tile_pool(name="psS", bufs=1, space="PSUM") as psSp:

                    # prologue: pair-0 projection, alternating PSUM pools so
                    # the copies pipeline; remaining xT chunks DMA behind
                    # the pair-0 weights
                    steps0 = make_proj(0, pools=[(psA, "psA"),
                                                 (psSp, "psS")])
                    for c in (1, 2, 3):
                        dma_xtc(c)
                    for step in steps0:
                        step()

                    for ht in range(NP):
                        if ht == 4:
                            load_tail_weights()
                        if ht >= 1:
                            emit_normalize(ht - 1)
                        QT, KTp = QK[ht]
                        po = [pavd.tile([128, 512], F32, name=f"po{qb}",
                                        tag=f"po{qb}") for qb in range(2)]
                        pd = pavd.tile([128, 512], F32, name="pd", tag="pd")
                        PTk = [ptp.tile([128, 2 * NQ], BF16, name=f"PT{k}",
                                        tag=f"PT{k}") for k in range(NT)]

                        if ht == 0:
                            v_steps.extend(make_v_panel(0))
                        if ht == 1:
                            v_steps.extend(make_v_panel(1))
                        if ht == 3:
                            v_steps.extend(make_v_panel(2))
                        if ht + 1 < NP:
                            v_steps.extend(make_proj(ht + 1))

                        n_done = 0      # av/denom slots emitted so far
                        deferred = []   # DVE-exp'd tiles awaiting av/denom
                        for k in range(NT):
                            pump(1)
                            psS = psSp.tile([128, 2 * NQ], F32, name="psS",
                                            tag="psS")
                            for hp in range(2):
                                hs = slice(hp * 64, (hp + 1) * 64)
                                for qb in range(2):
                                    qs = slice(qb * 512, (qb + 1) * 512)
                                    nc.tensor.matmul(
                                        psS[:, hp * NQ + qb * 512:
                                            hp * NQ + (qb + 1) * 512],
                                        KTp[hs, k * 128:(k + 1) * 128],
                                        QT[hs, qs],
                                        start=True, stop=True)
                                if hp == 0:
                                    pump(1)
                            emit_exp(psS, PTk[k], k)
                            deferred.append(k)
                            if k not in DVE_K:
                                while deferred:
                                    j = deferred.pop(0)
                                    av_step(ht, PTk, po, j, n_done)
                                    denom_step(PTk, pd, j, n_done)
                                    n_done += 1
                            pump(1)
                        while deferred:
                            j = deferred.pop(0)
                            av_step(ht, PTk, po, j, n_done)
                            denom_step(PTk, pd, j, n_done)
                            n_done += 1
                        pump(len(v_steps))

                        denom_combine(ht, pd)
                        av_drain(ht, po)

                        if debug_taps and ht == 0:
                            nc.sync.dma_start(taps["tap_V0"].ap(), V[0][:])
                            nc.sync.dma_start(taps["tap_QT0"].ap(), QT[:])
                            nc.sync.dma_start(taps["tap_KT0"].ap(), KTp[:])
                            nc.sync.dma_start(taps["tap_PTA4"].ap(),
                                              PTk[4][:, 0:NQ])
                            nc.sync.dma_start(taps["tap_PTA5"].ap(),
                                              PTk[5][:, 0:NQ])
                        if debug_taps and ht == 2:
                            nc.sync.dma_start(taps["tap_R0"].ap(), R[0][:])
                            nc.sync.dma_start(taps["tap_AOTU0"].ap(),
                                              AOTU[0][:])
                            nc.sync.dma_start(taps["tap_AOT0"].ap(),
                                              AOT[0][:])

                # ---- epilogue: last normalize + output projection ----
                with tc.tile_pool(name="psC", bufs=4, space="PSUM") as psC:
                    emit_normalize(NP - 1)

                    groups = [(qt, fo, fsz) for qt in range(QT_TILES)
                              for (fo, fsz) in ((0, 512), (512, 256))]
                    for (qt, fo, fsz) in groups:
                        ps = psC.tile([128, 512], F32, name="psF", tag="psF")
                        for i in range(DT):
                            nc.tensor.matmul(
                                ps[:, :fsz],
                                AOT[i][:, qt * 128:(qt + 1) * 128],
                                WOB[:, i * D + fo:i * D + fo + fsz],
                                start=(i == 0), stop=(i == DT - 1))
                        ot = outsp.tile([128, 512], F32, name="ot", tag="ot")
                        nc.vector.tensor_add(
                            ot[:, :fsz], ps[:, :fsz], BIAS[:, fo:fo + fsz])
                        nc.sync.dma_start(
                            out_d.ap()[qt * 128:(qt + 1) * 128, fo:fo + fsz],
                            ot[:, :fsz])

    nc.compile()
    return nc


_NC = None


def _get_nc():
    global _NC
    if _NC is None:
        _NC = build()
    return _NC


def make_in_maps(x, w_qkv, w_out, b_out):
    import ml_dtypes
    x = np.asarray(x, np.float32)
    w_qkv = np.ascontiguousarray(np.asarray(w_qkv, ml_dtypes.bfloat16))
    w_out = np.ascontiguousarray(np.asarray(w_out, ml_dtypes.bfloat16))
    bias = np.ascontiguousarray(
        np.broadcast_to(np.asarray(b_out, np.float32)[None, :], (128, D)))
    in_maps = []
    for c in range(N_CORES):
        b, half = divmod(c, 2)
        xb = x[b]
        qoff = half * NQ
        # query half first; key order permutation is harmless
        xperm = np.vstack([xb[qoff:qoff + NQ], xb[NQ - qoff:2 * NQ - qoff]])
        in_maps.append({
            "xT": np.ascontiguousarray(xperm.T.astype(ml_dtypes.bfloat16)),
            "wqkv": w_qkv,
            "wout": w_out,
            "bias": bias,
        })
    return in_maps


def run(in_maps, trace=False, **kw):
    return run_bass_kernel_spmd(_get_nc(), in_maps,
                                core_ids=list(range(N_CORES)),
                                trace=trace, **kw)


def assemble(results):
    out = np.empty((B, N, D), np.float32)
    for c in range(N_CORES):
        b, half = divmod(c, 2)
        out[b, half * NQ:(half + 1) * NQ, :] = results[c]["out"]
    return out


def kernel(x, w_qkv, w_out, b_out):
    res = run(make_in_maps(x, w_qkv, w_out, b_out))
    return assemble(res.results)


# revision 49
# speedup vs baseline: 1.2616x; 1.2616x over previous
"""Multi-head attention (B=4, N=2048, D=768, H=12, Dh=64) on 8 TRN2 NeuronCores.

Sharding: core c -> batch b = c//2, query rows half = c%2 (1024 rows each).
Each core computes all 12 heads for its (batch, query-half) against the full
2048-key sequence, so outputs are disjoint and no collective is needed.

Head-sequential eager pipeline: the kernel is a continuous stream of
"half-phases" (one per head). In half h, step k emits:
  - scores(h, k): two row-tiled K=64 matmuls (the head's 64 q/k dims live in
    array rows hp*64) -> S^T[128 keys, 1024 q] in a psS generation,
  - exp(h, k) on ACT (or, for k in DVE_K, a 2-instruction custom DVE op
    computing (1 + t + t^2/2)^2048 by repeated squaring),
  - attn@V of the PREVIOUS half at tile k: two [K=128, M=128, N=512] matmuls
    against the V panel (65 cols per head: 64 dims + ones column whose output
    row 64 is the softmax denominator),
  - one projection/V-panel filler step (next pair's Q^T/K^T, V panels).
The exp stream never waits on a serial prologue; the scalar engine runs
near-continuously.  Denominator rows are normalized baseline-style: fast
reciprocal + two K<=1 selector matmuls broadcast 1/d to the head's partition
rows, then one multiply.
"""

import numpy as np

import concourse.bass as bass
import concourse.bacc as bacc
import concourse.mybir as mybir
import concourse.tile as tile
from concourse.bass_utils import run_bass_kernel_spmd

N_CORES = 8
B, N, D = 4, 2048, 768
H, DH = 12, 64
NQ = 1024           # query rows per core
COLS = 3 * D        # 2304 qkv columns
DT = D // 128       # 6 partition tiles of the model dim
NT = N // 128       # 16 key tiles
QT_TILES = NQ // 128
NP = DT             # 6 head pairs
VG = DH + 1         # 65: head group width in V (64 cols + ones)

F32 = mybir.dt.float32
BF16 = mybir.dt.bfloat16

# k-tiles whose exp runs on the DVE via the custom squaring ops
DVE_K = (5, 11)


# ---------------- custom DVE exp ----------------
# exp(0.125*s) = u^2048, u = 1 + t + t^2/2 = ((t+1)^2 + 1)/2, t = 0.125*s/2048
def _register_exp_ops():
    import concourse.dve_ops as dve_ops
    from concourse.dve_spec import Spec, Src0, C0, C1, One, sq, lower
    from concourse.dve_uop import DveOpSpec
    from concourse.dve_table_gen import dve_ver_for

    def by_name(name):
        for o in dve_ops.OPS:
            if o.name == name:
                return o
        return None

    got = (by_name("EXP2K_BASE_ANT"), by_name("EXP2K_SQ8_ANT"))
    if got[0] is not None:
        return got

    a = sq(Src0 * C0 + One) + One
    body1 = sq(sq(sq(a * C1)))

    def ref1(in0, in1, s0, s1, imm2):
        u = ((in0.astype(np.float64) * s0 + 1.0) ** 2 + 1.0) * s1
        return (u ** 8).astype(np.float32)

    body2 = sq(sq(sq(sq(sq(sq(sq(sq(Src0))))))))

    def ref2(in0, in1, s0, s1, imm2):
        return (in0.astype(np.float64) ** 256).astype(np.float32)

    ver = dve_ver_for("TRN2")
    ops = []
    for name, body, ref in (("EXP2K_BASE_ANT", body1, ref1),
                            ("EXP2K_SQ8_ANT", body2, ref2)):
        spec = Spec(body=body, reference=ref)
        row = max(dve_ops._SUB_OPCODE_FOR_NAME.values()) + 1
        assert row < 0x20
        tmp = DveOpSpec(name=name, opcode=row, uops=lower(spec, ver=ver),
                        rd1_en=False)
        op = dve_ops.DveOp(name, spec, subdim=False,
                           uops_sha={ver: tmp.sha(ver)})
        dve_ops._SUB_OPCODE_FOR_NAME[name] = row
        dve_ops.OPS.append(op)
        dve_ops.CUSTOM_DVE_SPECS[name] = spec
        ops.append(op)
    return tuple(ops)


EXP_BASE, EXP_SQ8 = _register_exp_ops()


def build(debug_taps=False):
    nc = bacc.Bacc("TRN2", target_bir_lowering=False, debug=False,
                   num_devices=N_CORES)

    xT_d = nc.dram_tensor("xT", [D, N], BF16, kind="ExternalInput")
    wqkv_d = nc.dram_tensor("wqkv", [D, COLS], BF16, kind="ExternalInput")
    wout_d = nc.dram_tensor("wout", [D, D], BF16, kind="ExternalInput")
    bias_d = nc.dram_tensor("bias", [128, D], F32, kind="ExternalInput")
    out_d = nc.dram_tensor("out", [NQ, D], F32, kind="ExternalOutput")

    taps = {}
    if debug_taps:
        for name, shape, dt in (("tap_QT0", [128, NQ], BF16),
                                ("tap_KT0", [128, N], BF16),
                                ("tap_PTA4", [128, NQ], BF16),
                                ("tap_PTA5", [128, NQ], BF16)):
            taps[name] = nc.dram_tensor(name, shape, dt,
                                        kind="ExternalOutput")

    with tile.TileContext(nc) as tc:
        with tc.tile_pool(name="persist", bufs=1) as pp, \
             tc.tile_pool(name="small", bufs=2) as smallp, \
             tc.tile_pool(name="outs", bufs=3) as outsp:

            # V with a ones column per head and 63 pad cols so the attn@V
            # stationary can be a full 128-col slice
            V = [pp.tile([128, H * VG + 63], BF16, name=f"V{i}", tag=f"V{i}")
                 for i in range(NT)]
            AOT = [pp.tile([128, NQ], BF16, name=f"AOT{i}", tag=f"AOT{i}")
                   for i in range(NP)]
            WOB = pp.tile([128, DT * D], BF16, name="WOB", tag="WOB")
            BIAS = pp.tile([128, D], F32, name="BIAS", tag="BIAS")
            E1 = pp.tile([1, 128], BF16, name="E1", tag="E1")
            E2 = pp.tile([1, 128], BF16, name="E2", tag="E2")

            nc.gpsimd.memset(E1[:], 0.0)
            nc.gpsimd.memset(E2[:], 0.0)
            nc.gpsimd.memset(E1[0:1, 0:DH], 1.0)
            nc.gpsimd.memset(E2[0:1, DH:128], 1.0)

            def load_tail_weights():
                nc.sync.dma_start(BIAS[:], bias_d.ap())
                nc.sync.dma_start(
                    WOB[:].rearrange("p (a n) -> p a n", n=D),
                    wout_d.ap().rearrange("(a p) n -> p a n", p=128))

            with tc.tile_pool(name="projin", bufs=1) as projin, \
                 tc.tile_pool(name="qk", bufs=1) as qkp, \
                 tc.tile_pool(name="pt", bufs=1) as ptp, \
                 tc.tile_pool(name="scr", bufs=1) as scrp, \
                 tc.tile_pool(name="psA", bufs=2, space="PSUM") as psA, \
                 tc.tile_pool(name="po", bufs=1, space="PSUM") as pop:

                xTC = [projin.tile([128, DT * 512], BF16, name=f"xTC{c}",
                                   tag=f"xTC{c}") for c in range(4)]

                def dma_xtc(c):
                    nc.sync.dma_start(
                        xTC[c][:].rearrange("p (a n) -> p a n", n=512),
                        xT_d.ap()[:, c * 512:(c + 1) * 512].rearrange(
                            "(a p) n -> p a n", p=128))

                dma_xtc(0)

                def xT_ap(d, lo, hi):
                    c, off = divmod(lo, 512)
                    assert (hi - 1) // 512 == c
                    return xTC[c][:, d * 512 + off:d * 512 + off + (hi - lo)]

                # ---- filler steps (V panels + next-pair projections) ----
                v_steps = []

                def make_v_panel(vp):
                    co = 2 * D + vp * 256
                    wvB = projin.tile([128, DT * 256], BF16, name="wvB",
                                      tag="wvB", bufs=1)
                    wv = [wvB[:, d * 256:(d + 1) * 256] for d in range(DT)]
                    nc.sync.dma_start(
                        wvB[:].rearrange("p (a n) -> p a n", n=256),
                        wqkv_d.ap()[:, co:co + 256].rearrange(
                            "(a p) n -> p a n", p=128))

                    def step(t):
                        ps = psA.tile([128, 512], F32, name="psA", tag="psA")
                        for d in range(DT):
                            nc.tensor.matmul(
                                ps[:, :256],
                                xT_ap(d, t * 128, (t + 1) * 128),
                                wv[d],
                                start=(d == 0), stop=(d == DT - 1))
                        dst = V[t][:, 0:H * VG].rearrange(
                            "p (h c) -> p h c", c=VG)
                        nc.vector.tensor_copy(
                            dst[:, vp * 4:(vp + 1) * 4, 0:DH],
                            ps[:, :256].rearrange("p (h c) -> p h c", c=DH))
                        if vp == 0:
                            ones = V[t][:, 0:H * VG].rearrange(
                                "p (h c) -> p h c", c=VG)[:, :, DH:VG]
                            nc.gpsimd.memset(ones, 1.0)
                            nc.gpsimd.memset(V[t][:, H * VG:], 0.0)
                    return [lambda t=t: step(t) for t in range(NT)]

                QK = {}

                def make_proj(ht, pools=None):
                    QT = qkp.tile([128, NQ], BF16, name="QT", tag="QT",
                                  bufs=2)
                    KTp = qkp.tile([128, N], BF16, name="KTp", tag="KTp",
                                   bufs=2)
                    QK[ht] = (QT, KTp)
                    steps = []
                    nstep = [0]
                    for (dst, co, nn) in ((QT, ht * 128, NQ),
                                          (KTp, D + ht * 128, N)):
                        wpB = projin.tile([128, DT * 128], BF16,
                                          name="wqkB", tag="wqkB", bufs=2)
                        wp = [wpB[:, d * 128:(d + 1) * 128]
                              for d in range(DT)]
                        nc.sync.dma_start(
                            wpB[:].rearrange("p (a n) -> p a n", n=128),
                            wqkv_d.ap()[:, co:co + 128].rearrange(
                                "(a p) n -> p a n", p=128))

                        def nb_step(dst=dst, wp=wp, nb=0):
                            if pools is None:
                                ps = psA.tile([128, 512], F32, name="psA",
                                              tag="psA")
                            else:
                                pool, pname = pools[nstep[0] % len(pools)]
                                nstep[0] += 1
                                ps = pool.tile([128, 512], F32, name=pname,
                                               tag=pname)
                            for d in range(DT):
                                nc.tensor.matmul(
                                    ps[:],
                                    wp[d],
                                    xT_ap(d, nb * 512, (nb + 1) * 512),
                                    start=(d == 0), stop=(d == DT - 1))
                            nc.vector.tensor_copy(
                                dst[:, nb * 512:(nb + 1) * 512], ps[:])
                        for nb in range(nn // 512):
                            steps.append(lambda f=nb_step, nb=nb: f(nb=nb))
                    return steps

                def pump(n):
                    for _ in range(n):
                        if v_steps:
                            v_steps.pop(0)()

                # ---- attn@V slot for prev half at tile k ----
                def av_step(h, PTl, po, k):
                    ht, hp = divmod(h, 2)
                    for qb in range(2):
                        nc.tensor.matmul(
                            po[qb][:],
                            V[k][:, h * VG:h * VG + 128],
                            PTl[k][:, qb * 512:(qb + 1) * 512],
                            start=(k == 0), stop=(k == NT - 1))

                DD = {}  # h -> [1, NQ] denominator row

                def av_drain(h, po):
                    ht, hp = divmod(h, 2)
                    DD[h] = smallp.tile([1, NQ], F32, name=f"DD{h % 2}",
                                        tag=f"DD{h % 2}")
                    for qb in range(2):
                        qs = slice(qb * 512, (qb + 1) * 512)
                        nc.vector.tensor_copy(
                            AOT[ht][hp * DH:(hp + 1) * DH, qs],
                            po[qb][0:DH, :])
                        nc.vector.tensor_copy(DD[h][0:1, qs],
                                              po[qb][VG - 1:VG, :])

                def normalize_pair(ht):
                    RB = []
                    for hp in range(2):
                        rf = smallp.tile([1, NQ], F32, name=f"Rf{hp}",
                                         tag=f"Rf{hp}", bufs=1)
                        rb = smallp.tile([1, NQ], BF16, name=f"Rb{hp}",
                                         tag=f"Rb{hp}", bufs=1)
                        nc.vector.reciprocal_approx_fast(rf[:],
                                                         DD[2 * ht + hp][:])
                        nc.vector.tensor_copy(rb[:], rf[:])
                        RB.append(rb)
                    for qb in range(2):
                        qs = slice(qb * 512, (qb + 1) * 512)
                        rbp = psA.tile([128, 512], F32, name="psA",
                                       tag="psA")
                        nc.tensor.matmul(rbp[:], E1[:], RB[0][:, qs],
                                         start=True, stop=False)
                        nc.tensor.matmul(rbp[:], E2[:], RB[1][:, qs],
                                         start=False, stop=True)
                        nc.vector.tensor_mul(
                            AOT[ht][:, qs], AOT[ht][:, qs], rbp[:])

                def emit_exp(psS, PTtile, k):
                    if k in DVE_K:
                        scr = scrp.tile([128, NQ], F32, name="scr",
                                        tag="scr")
                        nc.vector._custom_dve(EXP_BASE, out=scr[:],
                                              in0=psS[:],
                                              s0=0.125 / 2048.0, s1=0.5)
                        nc.vector._custom_dve(EXP_SQ8, out=PTtile[:],
                                              in0=scr[:])
                    else:
                        nc.scalar.activation(
                            PTtile[:], psS[:],
                            mybir.ActivationFunctionType.Exp, scale=0.125)

                # ================= half-phases =================
                with tc.tile_pool(name="psS", bufs=2, space="PSUM") as psSp:
                    steps0 = make_proj(0, pools=[(psA, "psA"),
                                                 (psSp, "psS")])
                    for c in (1, 2, 3):
                        dma_xtc(c)
                    for step in steps0:
                        step()

                    prev = None  # (h, PTl, po) of the half in flight

                    for h in range(2 * NP):
                        ht, hp = divmod(h, 2)
                        if hp == 0:
                            if ht == 4:
                                load_tail_weights()
                            if ht >= 2:
                                normalize_pair(ht - 2)
                            if ht == 0:
                                v_steps.extend(make_v_panel(0))
                            if ht == 1:
                                v_steps.extend(make_v_panel(1))
                            if ht == 3:
                                v_steps.extend(make_v_panel(2))
                            if ht + 1 < NP:
                                v_steps.extend(make_proj(ht + 1))
                        QT, KTp = QK[ht]
                        hs = slice(hp * 64, (hp + 1) * 64)

                        if prev is not None:
                            po = [pop.tile([128, 512], F32, name=f"po{qb}",
                                           tag=f"po{qb}") for qb in range(2)]
                            prev = (prev[0], prev[1], po)
                        PTl = [ptp.tile([128, NQ], BF16, name=f"PT{hp}_{k}",
                                        tag=f"PT{hp}_{k}")
                               for k in range(NT)]

                        for k in range(NT):
                            pump(1)
                            psS = psSp.tile([128, NQ], F32, name="psS",
                                            tag="psS")
                            for qb in range(2):
                                qs = slice(qb * 512, (qb + 1) * 512)
                                nc.tensor.matmul(
                                    psS[:, qs],
                                    KTp[hs, k * 128:(k + 1) * 128],
                                    QT[hs, qs],
                                    start=True, stop=True)
                            emit_exp(psS, PTl[k], k)
                            if prev is not None:
                                av_step(prev[0], prev[1], prev[2], k)
                            pump(1)
                        if prev is not None:
                            av_drain(prev[0], prev[2])
                        prev = (h, PTl, None)

                        if debug_taps and h == 0:
                            nc.sync.dma_start(taps["tap_QT0"].ap(), QT[:])
                            nc.sync.dma_start(taps["tap_KT0"].ap(), KTp[:])
                            nc.sync.dma_start(taps["tap_PTA4"].ap(),
                                              PTl[4][:])
                            nc.sync.dma_start(taps["tap_PTA5"].ap(),
                                              PTl[5][:])

                # ---- epilogue: last half's attn@V, normalizes, out-proj ----
                with tc.tile_pool(name="psC", bufs=4, space="PSUM") as psC:
                    h, PTl, _ = prev
                    po = [pop.tile([128, 512], F32, name=f"po{qb}",
                                   tag=f"po{qb}") for qb in range(2)]
                    for k in range(NT):
                        av_step(h, PTl, po, k)
                    av_drain(h, po)
                    normalize_pair(NP - 2)
                    normalize_pair(NP - 1)

                    groups = [(qt, fo, fsz) for qt in range(QT_TILES)
                              for (fo, fsz) in ((0, 512), (512, 256))]
                    for (qt, fo, fsz) in groups:
                        ps = psC.tile([128, 512], F32, name="psF", tag="psF")
                        for i in range(DT):
                            nc.tensor.matmul(
                                ps[:, :fsz],
                                AOT[i][:, qt * 128:(qt + 1) * 128],
                                WOB[:, i * D + fo:i * D + fo + fsz],
                                start=(i == 0), stop=(i == DT - 1))
                        ot = outsp.tile([128, 512], F32, name="ot", tag="ot")
                        nc.vector.tensor_add(
                            ot[:, :fsz], ps[:, :fsz], BIAS[:, fo:fo + fsz])
                        nc.sync.dma_start(
                            out_d.ap()[qt * 128:(qt + 1) * 128, fo:fo + fsz],
                            ot[:, :fsz])

    nc.compile()
    return nc


_NC = None


def _get_nc():
    global _NC
    if _NC is None:
        _NC = build()
    return _NC


def make_in_maps(x, w_qkv, w_out, b_out):
    import ml_dtypes
    x = np.asarray(x, np.float32)
    w_qkv = np.ascontiguousarray(np.asarray(w_qkv, ml_dtypes.bfloat16))
    w_out = np.ascontiguousarray(np.asarray(w_out, ml_dtypes.bfloat16))
    bias = np.ascontiguousarray(
        np.broadcast_to(np.asarray(b_out, np.float32)[None, :], (128, D)))
    in_maps = []
    for c in range(N_CORES):
        b, half = divmod(c, 2)
        xb = x[b]
        qoff = half * NQ
        # query half first; key order permutation is harmless
        xperm = np.vstack([xb[qoff:qoff + NQ], xb[NQ - qoff:2 * NQ - qoff]])
        in_maps.append({
            "xT": np.ascontiguousarray(xperm.T.astype(ml_dtypes.bfloat16)),
            "wqkv": w_qkv,
            "wout": w_out,
            "bias": bias,
        })
    return in_maps


def run(in_maps, trace=False, **kw):
    return run_bass_kernel_spmd(_get_nc(), in_maps,
                                core_ids=list(range(N_CORES)),
                                trace=trace, **kw)


def assemble(results):
    out = np.empty((B, N, D), np.float32)
    for c in range(N_CORES):
        b, half = divmod(c, 2)
        out[b, half * NQ:(half + 1) * NQ, :] = results[c]["out"]
    return out


def kernel(x, w_qkv, w_out, b_out):
    res = run(make_in_maps(x, w_qkv, w_out, b_out))
    return assemble(res.results)


# revision 50
# speedup vs baseline: 1.7348x; 1.3751x over previous
"""Multi-head attention (B=4, N=2048, D=768, H=12, Dh=64) on 8 TRN2 NeuronCores.

Sharding: core c -> batch b = c//2, query rows half = c%2 (1024 rows each).
Each core computes all 12 heads for its (batch, query-half) against the full
2048-key sequence, so outputs are disjoint and no collective is needed.

Head-sequential eager pipeline: the kernel is a continuous stream of
"half-phases" (one per head). In half h, step k emits:
  - scores(h, k): two row-tiled K=64 matmuls (the head's 64 q/k dims live in
    array rows hp*64) -> S^T[128 keys, 1024 q] in a psS generation,
  - exp(h, k) on ACT (or, for k in DVE_K, a 2-instruction custom DVE op
    computing (1 + t + t^2/2)^2048 by repeated squaring),
  - attn@V of the PREVIOUS half at tile k: two [K=128, M=128, N=512] matmuls
    against the V panel (65 cols per head: 64 dims + ones column whose output
    row 64 is the softmax denominator),
  - one projection/V-panel filler step (next pair's Q^T/K^T, V panels).
The exp stream never waits on a serial prologue; the scalar engine runs
near-continuously.  Denominator rows are normalized baseline-style: fast
reciprocal + two K<=1 selector matmuls broadcast 1/d to the head's partition
rows, then one multiply.
"""

import numpy as np

import concourse.bass as bass
import concourse.bacc as bacc
import concourse.mybir as mybir
import concourse.tile as tile
from concourse.bass_utils import run_bass_kernel_spmd

N_CORES = 8
B, N, D = 4, 2048, 768
H, DH = 12, 64
NQ = 1024           # query rows per core
COLS = 3 * D        # 2304 qkv columns
DT = D // 128       # 6 partition tiles of the model dim
NT = N // 128       # 16 key tiles
QT_TILES = NQ // 128
NP = DT             # 6 head pairs
VG = DH + 1         # 65: head group width in V (64 cols + ones)

F32 = mybir.dt.float32
BF16 = mybir.dt.bfloat16

# k-tiles whose exp runs on the DVE via the custom squaring ops
DVE_K = (5, 11)


# ---------------- custom DVE exp ----------------
# exp(0.125*s) = u^2048, u = 1 + t + t^2/2 = ((t+1)^2 + 1)/2, t = 0.125*s/2048
def _register_exp_ops():
    import concourse.dve_ops as dve_ops
    from concourse.dve_spec import Spec, Src0, C0, C1, One, sq, lower
    from concourse.dve_uop import DveOpSpec
    from concourse.dve_table_gen import dve_ver_for

    def by_name(name):
        for o in dve_ops.OPS:
            if o.name == name:
                return o
        return None

    got = (by_name("EXP2K_BASE_ANT"), by_name("EXP2K_SQ8_ANT"))
    if got[0] is not None:
        return got

    a = sq(Src0 * C0 + One) + One
    body1 = sq(sq(sq(a * C1)))

    def ref1(in0, in1, s0, s1, imm2):
        u = ((in0.astype(np.float64) * s0 + 1.0) ** 2 + 1.0) * s1
        return (u ** 8).astype(np.float32)

    body2 = sq(sq(sq(sq(sq(sq(sq(sq(Src0))))))))

    def ref2(in0, in1, s0, s1, imm2):
        return (in0.astype(np.float64) ** 256).astype(np.float32)

    ver = dve_ver_for("TRN2")
    ops = []
    for name, body, ref in (("EXP2K_BASE_ANT", body1, ref1),
                            ("EXP2K_SQ8_ANT", body2, ref2)):
        spec = Spec(body=body, reference=ref)
        row = max(dve_ops._SUB_OPCODE_FOR_NAME.values()) + 1
        assert row < 0x20
        tmp = DveOpSpec(name=name, opcode=row, uops=lower(spec, ver=ver),
                        rd1_en=False)
        op = dve_ops.DveOp(name, spec, subdim=False,
                           uops_sha={ver: tmp.sha(ver)})
        dve_ops._SUB_OPCODE_FOR_NAME[name] = row
        dve_ops.OPS.append(op)
        dve_ops.CUSTOM_DVE_SPECS[name] = spec
        ops.append(op)
    return tuple(ops)


EXP_BASE, EXP_SQ8 = _register_exp_ops()


def build(debug_taps=False):
    nc = bacc.Bacc("TRN2", target_bir_lowering=False, debug=False,
                   num_devices=N_CORES)

    xT_d = nc.dram_tensor("xT", [D, N], BF16, kind="ExternalInput")
    wqkv_d = nc.dram_tensor("wqkv", [D, COLS], BF16, kind="ExternalInput")
    wout_d = nc.dram_tensor("wout", [D, D], BF16, kind="ExternalInput")
    bias_d = nc.dram_tensor("bias", [128, D], F32, kind="ExternalInput")
    out_d = nc.dram_tensor("out", [NQ, D], F32, kind="ExternalOutput")

    taps = {}
    if debug_taps:
        for name, shape, dt in (("tap_QT0", [128, NQ], BF16),
                                ("tap_KT0", [128, N], BF16),
                                ("tap_PTA4", [128, NQ], BF16),
                                ("tap_PTA5", [128, NQ], BF16)):
            taps[name] = nc.dram_tensor(name, shape, dt,
                                        kind="ExternalOutput")

    with tile.TileContext(nc) as tc:
        with tc.tile_pool(name="persist", bufs=1) as pp, \
             tc.tile_pool(name="small", bufs=2) as smallp, \
             tc.tile_pool(name="outs", bufs=3) as outsp:

            # V with a ones column per head and 63 pad cols so the attn@V
            # stationary can be a full 128-col slice
            V = [pp.tile([128, H * VG + 63], BF16, name=f"V{i}", tag=f"V{i}")
                 for i in range(NT)]
            AOT = [pp.tile([128, NQ], BF16, name=f"AOT{i}", tag=f"AOT{i}")
                   for i in range(NP)]
            WOB = pp.tile([128, DT * D], BF16, name="WOB", tag="WOB")
            BIAS = pp.tile([128, D], F32, name="BIAS", tag="BIAS")
            E1 = pp.tile([1, 128], BF16, name="E1", tag="E1")
            E2 = pp.tile([1, 128], BF16, name="E2", tag="E2")

            nc.gpsimd.memset(E1[:], 0.0)
            nc.gpsimd.memset(E2[:], 0.0)
            nc.gpsimd.memset(E1[0:1, 0:DH], 1.0)
            nc.gpsimd.memset(E2[0:1, DH:128], 1.0)

            def load_tail_weights():
                nc.sync.dma_start(BIAS[:], bias_d.ap())
                nc.sync.dma_start(
                    WOB[:].rearrange("p (a n) -> p a n", n=D),
                    wout_d.ap().rearrange("(a p) n -> p a n", p=128))

            with tc.tile_pool(name="projin", bufs=1) as projin, \
                 tc.tile_pool(name="qk", bufs=1) as qkp, \
                 tc.tile_pool(name="pt", bufs=1) as ptp, \
                 tc.tile_pool(name="scr", bufs=1) as scrp, \
                 tc.tile_pool(name="psA", bufs=2, space="PSUM") as psA, \
                 tc.tile_pool(name="po", bufs=1, space="PSUM") as pop:

                xTC = [projin.tile([128, DT * 512], BF16, name=f"xTC{c}",
                                   tag=f"xTC{c}") for c in range(4)]

                def dma_xtc(c):
                    nc.sync.dma_start(
                        xTC[c][:].rearrange("p (a n) -> p a n", n=512),
                        xT_d.ap()[:, c * 512:(c + 1) * 512].rearrange(
                            "(a p) n -> p a n", p=128))

                dma_xtc(0)

                def xT_ap(d, lo, hi):
                    c, off = divmod(lo, 512)
                    assert (hi - 1) // 512 == c
                    return xTC[c][:, d * 512 + off:d * 512 + off + (hi - lo)]

                # ---- filler steps (V panels + next-pair projections) ----
                v_steps = []

                def make_v_panel(vp):
                    co = 2 * D + vp * 256
                    wvB = projin.tile([128, DT * 256], BF16, name="wvB",
                                      tag="wvB", bufs=1)
                    wv = [wvB[:, d * 256:(d + 1) * 256] for d in range(DT)]
                    nc.sync.dma_start(
                        wvB[:].rearrange("p (a n) -> p a n", n=256),
                        wqkv_d.ap()[:, co:co + 256].rearrange(
                            "(a p) n -> p a n", p=128))

                    def step(t):
                        ps = psA.tile([128, 512], F32, name="psA", tag="psA")
                        for d in range(DT):
                            nc.tensor.matmul(
                                ps[:, :256],
                                xT_ap(d, t * 128, (t + 1) * 128),
                                wv[d],
                                start=(d == 0), stop=(d == DT - 1))
                        dst = V[t][:, 0:H * VG].rearrange(
                            "p (h c) -> p h c", c=VG)
                        nc.vector.tensor_copy(
                            dst[:, vp * 4:(vp + 1) * 4, 0:DH],
                            ps[:, :256].rearrange("p (h c) -> p h c", c=DH))
                        if vp == 0:
                            ones = V[t][:, 0:H * VG].rearrange(
                                "p (h c) -> p h c", c=VG)[:, :, DH:VG]
                            nc.gpsimd.memset(ones, 1.0)
                            nc.gpsimd.memset(V[t][:, H * VG:], 0.0)
                    return [lambda t=t: step(t) for t in range(NT)]

                QK = {}

                def make_proj(ht, pools=None):
                    QZ = [qkp.tile([128, NQ], BF16, name=f"QZ{j}",
                                   tag=f"QZ{j}", bufs=2) for j in range(2)]
                    KTp = qkp.tile([128, N], BF16, name="KTp", tag="KTp",
                                   bufs=2)
                    QK[ht] = (QZ, KTp)
                    nc.gpsimd.memset(QZ[0][DH:128, :], 0.0)
                    nc.gpsimd.memset(QZ[1][0:DH, :], 0.0)
                    steps = []
                    nstep = [0]
                    for (dst, co, nn) in ((None, ht * 128, NQ),
                                          (KTp, D + ht * 128, N)):
                        wpB = projin.tile([128, DT * 128], BF16,
                                          name="wqkB", tag="wqkB", bufs=2)
                        wp = [wpB[:, d * 128:(d + 1) * 128]
                              for d in range(DT)]
                        nc.sync.dma_start(
                            wpB[:].rearrange("p (a n) -> p a n", n=128),
                            wqkv_d.ap()[:, co:co + 128].rearrange(
                                "(a p) n -> p a n", p=128))

                        def nb_step(dst=dst, wp=wp, nb=0):
                            if pools is None:
                                ps = psA.tile([128, 512], F32, name="psA",
                                              tag="psA")
                            else:
                                pool, pname = pools[nstep[0] % len(pools)]
                                nstep[0] += 1
                                ps = pool.tile([128, 512], F32, name=pname,
                                               tag=pname)
                            for d in range(DT):
                                nc.tensor.matmul(
                                    ps[:],
                                    wp[d],
                                    xT_ap(d, nb * 512, (nb + 1) * 512),
                                    start=(d == 0), stop=(d == DT - 1))
                            nbs = slice(nb * 512, (nb + 1) * 512)
                            if dst is not None:
                                nc.vector.tensor_copy(dst[:, nbs], ps[:])
                            else:
                                nc.vector.tensor_copy(
                                    QZ[0][0:DH, nbs], ps[0:DH, :])
                                nc.vector.tensor_copy(
                                    QZ[1][DH:128, nbs], ps[DH:128, :])
                        for nb in range(nn // 512):
                            steps.append(lambda f=nb_step, nb=nb: f(nb=nb))
                    return steps

                def pump(n):
                    for _ in range(n):
                        if v_steps:
                            v_steps.pop(0)()

                # ---- attn@V slot for prev half at tile k ----
                def av_step(h, PTl, po, k):
                    ht, hp = divmod(h, 2)
                    for qb in range(2):
                        nc.tensor.matmul(
                            po[qb][:],
                            V[k][:, h * VG:h * VG + 128],
                            PTl[k][:, qb * 512:(qb + 1) * 512],
                            start=(k == 0), stop=(k == NT - 1))

                DD = {}  # h -> [1, NQ] denominator row

                def av_drain(h, po):
                    ht, hp = divmod(h, 2)
                    DD[h] = smallp.tile([1, NQ], F32, name=f"DD{h % 2}",
                                        tag=f"DD{h % 2}")
                    for qb in range(2):
                        qs = slice(qb * 512, (qb + 1) * 512)
                        nc.vector.tensor_copy(
                            AOT[ht][hp * DH:(hp + 1) * DH, qs],
                            po[qb][0:DH, :])
                        nc.vector.tensor_copy(DD[h][0:1, qs],
                                              po[qb][VG - 1:VG, :])

                def normalize_pair(ht):
                    RB = []
                    for hp in range(2):
                        rf = smallp.tile([1, NQ], F32, name=f"Rf{hp}",
                                         tag=f"Rf{hp}", bufs=1)
                        rb = smallp.tile([1, NQ], BF16, name=f"Rb{hp}",
                                         tag=f"Rb{hp}", bufs=1)
                        nc.vector.reciprocal_approx_fast(rf[:],
                                                         DD[2 * ht + hp][:])
                        nc.vector.tensor_copy(rb[:], rf[:])
                        RB.append(rb)
                    for qb in range(2):
                        qs = slice(qb * 512, (qb + 1) * 512)
                        rbp = psA.tile([128, 512], F32, name="psA",
                                       tag="psA")
                        nc.tensor.matmul(rbp[:], E1[:], RB[0][:, qs],
                                         start=True, stop=False)
                        nc.tensor.matmul(rbp[:], E2[:], RB[1][:, qs],
                                         start=False, stop=True)
                        nc.vector.tensor_mul(
                            AOT[ht][:, qs], AOT[ht][:, qs], rbp[:])

                def emit_exp(psS, PTtile, k):
                    if k in DVE_K:
                        scr = scrp.tile([128, NQ], F32, name="scr",
                                        tag="scr")
                        nc.vector._custom_dve(EXP_BASE, out=scr[:],
                                              in0=psS[:],
                                              s0=0.125 / 2048.0, s1=0.5)
                        nc.vector._custom_dve(EXP_SQ8, out=PTtile[:],
                                              in0=scr[:])
                    else:
                        nc.scalar.activation(
                            PTtile[:], psS[:],
                            mybir.ActivationFunctionType.Exp, scale=0.125)

                # ================= half-phases =================
                with tc.tile_pool(name="psS", bufs=2, space="PSUM") as psSp:
                    steps0 = make_proj(0, pools=[(psA, "psA"),
                                                 (psSp, "psS")])
                    for c in (1, 2, 3):
                        dma_xtc(c)
                    for step in steps0:
                        step()

                    prev = None  # (h, PTl, po) of the half in flight

                    for h in range(2 * NP):
                        ht, hp = divmod(h, 2)
                        if hp == 0:
                            if ht == 4:
                                load_tail_weights()
                            if ht >= 2:
                                normalize_pair(ht - 2)
                            if ht == 0:
                                v_steps.extend(make_v_panel(0))
                            if ht == 1:
                                v_steps.extend(make_v_panel(1))
                            if ht == 3:
                                v_steps.extend(make_v_panel(2))
                            if ht + 1 < NP:
                                v_steps.extend(make_proj(ht + 1))
                        QZ, KTp = QK[ht]

                        if prev is not None:
                            po = [pop.tile([128, 512], F32, name=f"po{qb}",
                                           tag=f"po{qb}") for qb in range(2)]
                            prev = (prev[0], prev[1], po)
                        PTl = [ptp.tile([128, NQ], BF16, name=f"PT{hp}_{k}",
                                        tag=f"PT{hp}_{k}")
                               for k in range(NT)]

                        for k in range(NT):
                            pump(1)
                            psS = psSp.tile([128, NQ], F32, name="psS",
                                            tag="psS")
                            for qb in range(2):
                                qs = slice(qb * 512, (qb + 1) * 512)
                                nc.tensor.matmul(
                                    psS[:, qs],
                                    KTp[:, k * 128:(k + 1) * 128],
                                    QZ[hp][:, qs],
                                    start=True, stop=True)
                            emit_exp(psS, PTl[k], k)
                            if prev is not None:
                                av_step(prev[0], prev[1], prev[2], k)
                            pump(1)
                        if prev is not None:
                            av_drain(prev[0], prev[2])
                        prev = (h, PTl, None)

                        if debug_taps and h == 0:
                            nc.sync.dma_start(taps["tap_QT0"].ap(), QZ[0][:])
                            nc.sync.dma_start(taps["tap_KT0"].ap(), KTp[:])
                            nc.sync.dma_start(taps["tap_PTA4"].ap(),
                                              PTl[4][:])
                            nc.sync.dma_start(taps["tap_PTA5"].ap(),
                                              PTl[5][:])

                # ---- epilogue: last half's attn@V, normalizes, out-proj ----
                with tc.tile_pool(name="psC", bufs=4, space="PSUM") as psC:
                    h, PTl, _ = prev
                    po = [pop.tile([128, 512], F32, name=f"po{qb}",
                                   tag=f"po{qb}") for qb in range(2)]
                    for k in range(NT):
                        av_step(h, PTl, po, k)
                    av_drain(h, po)
                    normalize_pair(NP - 2)
                    normalize_pair(NP - 1)

                    groups = [(qt, fo, fsz) for qt in range(QT_TILES)
                              for (fo, fsz) in ((0, 512), (512, 256))]
                    for (qt, fo, fsz) in groups:
                        ps = psC.tile([128, 512], F32, name="psF", tag="psF")
                        for i in range(DT):
                            nc.tensor.matmul(
                                ps[:, :fsz],
                                AOT[i][:, qt * 128:(qt + 1) * 128],
                                WOB[:, i * D + fo:i * D + fo + fsz],
                                start=(i == 0), stop=(i == DT - 1))
                        ot = outsp.tile([128, 512], F32, name="ot", tag="ot")
                        nc.vector.tensor_add(
                            ot[:, :fsz], ps[:, :fsz], BIAS[:, fo:fo + fsz])
                        nc.sync.dma_start(
                            out_d.ap()[qt * 128:(qt + 1) * 128, fo:fo + fsz],
                            ot[:, :fsz])

    nc.compile()
    return nc


_NC = None


def _get_nc():
    global _NC
    if _NC is None:
        _NC = build()
    return _NC


def make_in_maps(x, w_qkv, w_out, b_out):
    import ml_dtypes
    x = np.asarray(x, np.float32)
    w_qkv = np.ascontiguousarray(np.asarray(w_qkv, ml_dtypes.bfloat16))
    w_out = np.ascontiguousarray(np.asarray(w_out, ml_dtypes.bfloat16))
    bias = np.ascontiguousarray(
        np.broadcast_to(np.asarray(b_out, np.float32)[None, :], (128, D)))
    in_maps = []
    for c in range(N_CORES):
        b, half = divmod(c, 2)
        xb = x[b]
        qoff = half * NQ
        # query half first; key order permutation is harmless
        xperm = np.vstack([xb[qoff:qoff + NQ], xb[NQ - qoff:2 * NQ - qoff]])
        in_maps.append({
            "xT": np.ascontiguousarray(xperm.T.astype(ml_dtypes.bfloat16)),
            "wqkv": w_qkv,
            "wout": w_out,
            "bias": bias,
        })
    return in_maps


def run(in_maps, trace=False, **kw):
    return run_bass_kernel_spmd(_get_nc(), in_maps,
                                core_ids=list(range(N_CORES)),
                                trace=trace, **kw)


def assemble(results):
    out = np.empty((B, N, D), np.float32)
    for c in range(N_CORES):
        b, half = divmod(c, 2)
        out[b, half * NQ:(half + 1) * NQ, :] = results[c]["out"]
    return out


def kernel(x, w_qkv, w_out, b_out):
    res = run(make_in_maps(x, w_qkv, w_out, b_out))
    return assemble(res.results)
